# revision 23
# baseline (speedup 1.0000x reference)
"""Trainium2 Bass kernel for ChebConv(K=2) x2 + Linear GNN.

Sharding: nodes are sharded over 8 cores by destination (25000/core); edges
are partitioned by dst shard. Per core, local nodes are relabeled by
in-degree (desc), so "round r" (each dst's r-th incoming edge) is a prefix
of the local rank space. Gather+scatter-add is implemented as per-round
multi-column indirect-DMA gathers from a replicated source table with
CCE-add accumulation into an SBUF accumulator (round 0 overwrites; pad
slots gather a zero row). Each indirect DMA covers a whole (subblock,
round) span, amortizing the ~1us SWDGE fixed cost over up to 28 columns.

Math identity: with s = sqrt(1/max(deg,1)) (deg = out-degree; s==1 where
deg==0 but those rows are never gathered since they are never edge
sources) and nd = -min(deg,1)*s:
    P(h) = segment_sum(-dis[src]*dis[dst]*h[src], dst)
         = nd * segment_sum((s*h)[src], dst)
so per-edge weights never materialize: gather tables store s*h rows and
the accumulator is scaled by nd afterwards.

Pipeline (host does only layout: pad/permute/transpose/concat/casts):
  A:  s, nd, y = s*x                       (sharded by node, orig order)
  BC: gather-add y table -> Px = nd*acc; PE-transpose to feature-major;
      h1 = relu([x|1]@[W1_0;b1] + Px@W1_1); write h1 and s*h1 (rank order)
  DE: gather-add (s*h1) table -> Ph = nd*acc; PE-transpose; feature-major
      h2 = relu(W2_0^T h1T + W2_1^T PhT + b2); out = h2@Wl + bl
      (node-major Wl matmuls, ap=2)
"""
import numpy as np
import ml_dtypes

BF = ml_dtypes.bfloat16

N = 200000
E = 400000
F = 165
H = 512
C = 2
NCORES = 8
NLOC = N // NCORES          # 25000
P = 128
CH = (NLOC + P - 1) // P    # 196
NLOCP = CH * P              # 25088
ZROW = N                    # zero row index in gather tables
FA = F + 1                  # 166 (x augmented with ones)
FP = 256                    # x gather table row width (padded to 512B rows)

_CACHE = {}


# ----------------------------------------------------------------------------
# host-side index prep (pure integer/layout work)
# ----------------------------------------------------------------------------

def _host_prep(src, dst):
    indeg = np.bincount(dst, minlength=N)
    perms = []          # per core: global node ids in rank order [NLOC]
    srcs_rounds = []    # per core: list over r of np.ndarray (len N_r)
    for p in range(NCORES):
        lo = p * NLOC
        indeg_l = indeg[lo:lo + NLOC]
        order = np.argsort(-indeg_l, kind="stable")
        perms.append(lo + order)
        rank_of = np.empty(NLOC, np.int64)
        rank_of[order] = np.arange(NLOC)
        em = (dst >= lo) & (dst < lo + NLOC)
        es, ed = src[em], dst[em]
        dr = rank_of[ed - lo]
        o2 = np.argsort(dr, kind="stable")
        es, dr = es[o2], dr[o2]
        # position within each dst run
        n = len(dr)
        first = np.ones(n, bool)
        first[1:] = dr[1:] != dr[:-1]
        runstart = np.maximum.accumulate(np.where(first, np.arange(n), 0))
        pos = np.arange(n) - runstart
        rmax = int(indeg_l.max()) if n else 0
        rounds = []
        for r in range(rmax):
            sel = pos == r
            rounds.append(es[sel].astype(np.int64))  # aligned to ranks 0..N_r-1
        srcs_rounds.append(rounds)

    R = max(1, max(len(r) for r in srcs_rounds))
    ks = []
    for r in range(R):
        if r == 0:
            n1 = max((len(sr[0]) if sr else 0) for sr in srcs_rounds)
            ks.append(min(CH, max(1, (n1 + P - 1) // P)))
        else:
            nr = max((len(sr[r]) if r < len(sr) else 0) for sr in srcs_rounds)
            ks.append(max(1, (nr + P - 1) // P))
    K = sum(ks)

    idx = np.full((NCORES, P, K), ZROW, np.int32)
    j0 = 0
    for r, k in enumerate(ks):
        for p in range(NCORES):
            sr = srcs_rounds[p][r] if r < len(srcs_rounds[p]) else np.empty(0, np.int64)
            buf = np.full(k * P, ZROW, np.int64)
            buf[: len(sr)] = sr
            idx[p, :, j0:j0 + k] = buf.reshape(k, P).T
        j0 += k
    return perms, idx, tuple(ks)


def _cmajor(v):
    """[NLOCP] -> [P, CH] with [i, c] = v[c*P + i]."""
    return np.ascontiguousarray(v.reshape(CH, P).T)


# ----------------------------------------------------------------------------
# infra: walrus-wait-limit workarounds + SPMD runner (self-contained)
# ----------------------------------------------------------------------------

class _Infra:
    applied = False

    @staticmethod
    def apply():
        if _Infra.applied:
            return
        import concourse.tile as tile_mod
        import concourse.mybir as mybir
        from concourse.vector_clock import ScopedClock

        def _patched_drain_and_barrier(self, tick_clock, wait_clock):
            nop0 = self.nc.sync.nop(nofuse=True)
            wait_clock.add_sem_waits(nop0.ins, ScopedClock({None: tick_clock.global_clock}))
            si = nop0.ins.sync_info
            waits = list(si.on_wait) if si is not None else []
            if len(waits) > 1:
                si.on_wait[:] = waits[:1]
                for i in range(1, len(waits)):
                    nop = self.nc.sync.nop(nofuse=True)
                    nsi = nop.ins.sync_info
                    if nsi is None:
                        nop.ins.sync_info = mybir.SyncInfo(
                            on_wait=[waits[i]], on_update=[])
                    else:
                        nsi.on_wait[:] = [waits[i]]
            self.nc.sync.drain()
            self.nc.all_engine_barrier()
            assert self.sems is not None
            popped = self.nc._tile_sem_poison_stack.pop()
            assert popped is self._sem_poison
            self.nc.clear_and_free_semaphores(list(self.sems.allocated().values()))
            self.nc.all_engine_barrier()

        tile_mod.TileContext._drain_and_barrier = _patched_drain_and_barrier
        _Infra.applied = True

    @staticmethod
    def legalize_waits(nc, maxw=1):
        import concourse.mybir as mybir
        n_added = 0
        for fn in nc.m.functions:
            for blk in fn.blocks:
                out = []
                for inst in blk.instructions:
                    si = inst.sync_info
                    if si is not None and len(si.on_wait) > maxw:
                        waits = list(si.on_wait)
                        si.on_wait[:] = waits[:maxw]
                        rest = waits[maxw:]
                        for i in range(0, len(rest), maxw):
                            nop = mybir.InstNoOp(
                                name=f"{inst.name}-lw{i}", ins=[], outs=[])
                            nop.engine = inst.engine
                            nop.sync_info = mybir.SyncInfo(
                                on_wait=rest[i:i + maxw], on_update=[])
                            out.append(nop)
                            n_added += 1
                    out.append(inst)
                blk.instructions[:] = out
        return n_added


class SpmdKernel:
    """Compile a Bass program once; run it SPMD on 8 cores via PJRT."""

    def __init__(self, nc, n_cores=8):
        import jax
        import concourse.mybir as mybir
        from jax.sharding import Mesh, PartitionSpec
        from jax.experimental.shard_map import shard_map
        from concourse.bass2jax import (
            _bass_exec_p, install_neuronx_cc_hook, partition_id_tensor)
        install_neuronx_cc_hook()
        self.nc = nc
        self.n_cores = n_cores
        in_names, out_names, out_avals = [], [], []
        partition_name = nc.partition_id_tensor.name if nc.partition_id_tensor else None
        for alloc in nc.m.functions[0].allocations:
            if not isinstance(alloc, mybir.MemoryLocationSet):
                continue
            name = alloc.memorylocations[0].name
            if alloc.kind == "ExternalInput":
                if name != partition_name:
                    in_names.append(name)
            elif alloc.kind == "ExternalOutput":
                out_names.append(name)
                out_avals.append(jax.core.ShapedArray(
                    tuple(alloc.tensor_shape), mybir.dt.np(alloc.dtype)))
        self.in_names, self.out_names, self.out_avals = in_names, out_names, out_avals
        all_in_names = list(in_names) + list(out_names)
        if partition_name is not None:
            all_in_names.append(partition_name)

        def _body(*args):
            operands = list(args)
            if partition_name is not None:
                operands.append(partition_id_tensor())
            outs = _bass_exec_p.bind(
                *operands,
                out_avals=tuple(out_avals),
                in_names=tuple(all_in_names),
                out_names=tuple(out_names),
                lowering_input_output_aliases=(),
                sim_require_finite=False,
                sim_require_nnan=False,
                nc=nc,
            )
            return tuple(outs)

        devices = jax.devices()[:n_cores]
        self.mesh = Mesh(np.asarray(devices), ("core",))
        in_specs = (PartitionSpec("core"),) * (len(in_names) + len(out_names))
        out_specs = (PartitionSpec("core"),) * len(out_names)
        self.fn = jax.jit(
            shard_map(_body, mesh=self.mesh, in_specs=in_specs,
                      out_specs=out_specs, check_rep=False),
            keep_unused=True,
        )
        self.sharding = jax.sharding.NamedSharding(self.mesh, PartitionSpec("core"))
        self._jax = jax

    def place(self, in_maps):
        jax = self._jax
        placed = []
        for name in self.in_names:
            concat = np.concatenate([np.asarray(m[name]) for m in in_maps], axis=0)
            placed.append(jax.device_put(concat, self.sharding))
        for av in self.out_avals:
            z = np.zeros((self.n_cores * av.shape[0], *av.shape[1:]), av.dtype)
            placed.append(jax.device_put(z, self.sharding))
        return placed

    def run(self, placed):
        outs = [np.asarray(o) for o in self.fn(*placed)]
        res = []
        for c in range(self.n_cores):
            d = {}
            for i, name in enumerate(self.out_names):
                shp = self.out_avals[i].shape
                d[name] = outs[i].reshape(self.n_cores, *shp)[c]
            res.append(d)
        return res

    def time_iters(self, placed, iters=8, warmup=2):
        import time as _time
        jax = self._jax
        r = None
        for _ in range(warmup):
            r = self.fn(*placed)
        jax.block_until_ready(r)
        t0 = _time.perf_counter()
        outs = None
        for _ in range(iters):
            outs = self.fn(*placed)
        jax.block_until_ready(outs)
        return (_time.perf_counter() - t0) / iters


def _get_mods():
    import concourse.bass as bass
    import concourse.mybir as mybir
    import concourse.tile as tile
    _Infra.apply()
    return bass, mybir, tile


# ----------------------------------------------------------------------------
# gather emission helper: one indirect DMA per (subblock, round) span
# ----------------------------------------------------------------------------

def _emit_gathers(nc, mybir, acc, table, idx, ks, c0s, c1s, width, acc_dt_sz=2):
    """Emit multi-column CCE gathers filling acc[:, 0:(c1s-c0s)*width] for
    chunk columns [c0s, c1s). Round 0 overwrites (bypass); later rounds
    CCE-add. Columns >= ks[0] are memset first."""
    from concourse.bass import IndirectOffsetOnAxis
    AL = mybir.AluOpType
    ms_lo = max(c0s, ks[0])
    if ms_lo < c1s:
        nc.vector.memset(acc[:, (ms_lo - c0s) * width:(c1s - c0s) * width], 0.0)
    j0 = 0
    for r, k in enumerate(ks):
        lo, hi = c0s, min(c1s, k)
        # one indirect DMA per 128-row column: the vector-indirect DGE only
        # supports a single dest run per partition (2D dest AP).
        for c in range(lo, hi):
            nc.gpsimd.indirect_dma_start(
                out=acc[:, (c - c0s) * width:(c - c0s + 1) * width],
                out_offset=None,
                in_=table[:],
                in_offset=IndirectOffsetOnAxis(
                    ap=idx[:, j0 + c:j0 + c + 1], axis=0),
                compute_op=(AL.bypass if r == 0 else AL.add),
            )
        j0 += k


# ----------------------------------------------------------------------------
# launch A: s, nd, y = s*x
# ----------------------------------------------------------------------------

def _build_A():
    bass, mybir, tile = _get_mods()
    nc = bass.Bass()
    BFD = mybir.dt.bfloat16
    # cmajor layout: xcm[p, c*F:(c+1)*F] = x[c*P + p, :]
    x_in = nc.declare_dram_parameter("xcm", [P, CH * F], BFD, isOutput=False)
    deg_in = nc.declare_dram_parameter("deg", [P, CH], mybir.dt.float32, isOutput=False)
    y_out = nc.declare_dram_parameter("ycm", [P, CH * F], BFD, isOutput=True)
    s_out = nc.declare_dram_parameter("s", [P, CH], mybir.dt.float32, isOutput=True)
    snd_out = nc.declare_dram_parameter("snd", [P, CH], mybir.dt.float32, isOutput=True)
    rs_out = nc.declare_dram_parameter("rs", [P, CH], mybir.dt.float32, isOutput=True)
    AL = mybir.AluOpType
    with tile.TileContext(nc) as tc:
        with tc.tile_pool(name="sb", bufs=3) as pool, \
             tc.tile_pool(name="cons", bufs=1) as cpool:
            deg = cpool.tile([P, CH], mybir.dt.float32)
            mask = cpool.tile([P, CH], mybir.dt.float32)
            rec = cpool.tile([P, CH], mybir.dt.float32)
            s = cpool.tile([P, CH], mybir.dt.float32)
            snd = cpool.tile([P, CH], mybir.dt.float32)
            rs = cpool.tile([P, CH], mybir.dt.float32)
            nc.sync.dma_start(out=deg[:], in_=deg_in[:])
            # s = sqrt(1/max(deg,1)); snd = -min(deg,1)*s^2; rs = 1/s
            nc.vector.tensor_scalar(mask[:], deg[:], 1.0, None, AL.min)
            nc.vector.tensor_scalar(rec[:], deg[:], 1.0, None, AL.max)
            nc.scalar.sqrt(rs[:], rec[:])
            nc.vector.reciprocal(rec[:], rec[:])
            nc.scalar.sqrt(s[:], rec[:])
            nc.vector.tensor_tensor(out=snd[:], in0=rec[:], in1=mask[:], op=AL.mult)
            nc.vector.tensor_scalar(snd[:], snd[:], -1.0, None, AL.mult)
            nc.sync.dma_start(out=s_out[:], in_=s[:])
            nc.sync.dma_start(out=snd_out[:], in_=snd[:])
            nc.sync.dma_start(out=rs_out[:], in_=rs[:])
            G = 14
            for c0 in range(0, CH, G):
                g = min(G, CH - c0)
                xt = pool.tile([P, G * F], BFD, tag="xt")
                yb = pool.tile([P, G * F], BFD, tag="yb")
                nc.sync.dma_start(out=xt[:, :g * F], in_=x_in[:, c0 * F:(c0 + g) * F])
                for j in range(g):
                    if j % 2 == 0:
                        nc.vector.tensor_scalar(
                            yb[:, j * F:(j + 1) * F], xt[:, j * F:(j + 1) * F],
                            s[:, c0 + j:c0 + j + 1], None, AL.mult)
                    else:
                        nc.scalar.activation(
                            yb[:, j * F:(j + 1) * F], xt[:, j * F:(j + 1) * F],
                            mybir.ActivationFunctionType.Copy,
                            scale=s[:, c0 + j:c0 + j + 1])
                nc.sync.dma_start(out=y_out[:, c0 * F:(c0 + g) * F], in_=yb[:, :g * F])
    _Infra.legalize_waits(nc)
    return SpmdKernel(nc, NCORES)


# ----------------------------------------------------------------------------
# launch BC: gather Px + layer-1 matmuls -> h1, s*h1 (rank order)
# ----------------------------------------------------------------------------

def _build_BC(ks, nsub=7, G=7):
    """Gather Px + layer-1 matmuls in the s-scaled domain -> sh1 = s*h1.

    Inputs are pre-scaled: yaugT = [ (s*x)^T ; s ], gather table holds s*x
    rows, and the gathered accumulator is transpose-scaled by diag(s*nd)
    (one plain matmul per 128-col block: out = acc^T @ diag). Since s>0,
    relu(s*pre1) == s*relu(pre1) == s*h1.

    Software-pipelined: transposes+copies for group g are emitted before the
    matmuls of group g-1, so PSUM->SBUF copies overlap PE matmul work.
    """
    bass, mybir, tile = _get_mods()
    AL = mybir.AluOpType
    AF = mybir.ActivationFunctionType
    BFD = mybir.dt.bfloat16
    K = sum(ks)
    CHS = (CH + nsub - 1) // nsub
    k1a, k1b = P, FA - P      # 128 + 38
    k2a, k2b = P, F - P       # 128 + 37
    nc = bass.Bass()
    table = nc.declare_dram_parameter("table", [N + 1, FP], BFD, isOutput=False)
    idx_in = nc.declare_dram_parameter("idx", [P, K], mybir.dt.int32, isOutput=False)
    dg_in = nc.declare_dram_parameter("dg", [P, CH * P], BFD, isOutput=False)
    yaT = nc.declare_dram_parameter("yaugT", [FA, NLOCP], BFD, isOutput=False)
    w10 = nc.declare_dram_parameter("w10aug", [FA, H], BFD, isOutput=False)
    w11 = nc.declare_dram_parameter("w11", [F, H], BFD, isOutput=False)
    sh1_out = nc.declare_dram_parameter("sh1", [NLOCP, H], BFD, isOutput=True)

    class Grp:
        pass

    with tile.TileContext(nc) as tc:
        with tc.tile_pool(name="w", bufs=1) as wp, \
             tc.tile_pool(name="accp", bufs=2) as accp, \
             tc.tile_pool(name="dgp", bufs=2) as dgp, \
             tc.tile_pool(name="io", bufs=3) as io, \
             tc.tile_pool(name="ps", bufs=2, space="PSUM") as ps, \
             tc.tile_pool(name="pst", bufs=3, space="PSUM") as pst:
            idx = wp.tile([P, K], mybir.dt.int32)
            w10a = wp.tile([k1a, H], BFD)
            w10b = wp.tile([k1b, H], BFD)
            w11a = wp.tile([k2a, H], BFD)
            w11b = wp.tile([k2b, H], BFD)
            nc.sync.dma_start(out=idx[:], in_=idx_in[:])
            nc.sync.dma_start(out=w10a[:], in_=w10[0:k1a, :])
            nc.sync.dma_start(out=w10b[:], in_=w10[k1a:FA, :])
            nc.sync.dma_start(out=w11a[:], in_=w11[0:k2a, :])
            nc.sync.dma_start(out=w11b[:], in_=w11[k2a:F, :])

            def stage_front(sb, c0, g, acc, dgt):
                """Load inputs + emit transpose/copies for one group."""
                c0s = sb * CHS
                n0 = c0 * P
                nw = g * P
                fr = Grp()
                fr.c0, fr.g, fr.n0, fr.nw = c0, g, n0, nw
                fr.ya = io.tile([k1a, G * P], BFD, tag="ya")
                fr.yb = io.tile([k1b, G * P], BFD, tag="yb")
                nc.sync.dma_start(out=fr.ya[:, :nw], in_=yaT[0:k1a, n0:n0 + nw])
                nc.sync.dma_start(out=fr.yb[:, :nw], in_=yaT[k1a:FA, n0:n0 + nw])
                fr.pxa = io.tile([k2a, G * P], BFD, tag="pxa")
                fr.pxb = io.tile([k2b, G * P], BFD, tag="pxb")
                for j in range(g):
                    cl = c0 - c0s + j
                    jp = j * P
                    tps = pst.tile([P, 2 * P], mybir.dt.float32, tag="t1")
                    nc.tensor.matmul(
                        tps[:, 0:P],
                        lhsT=acc[:, cl * FP:cl * FP + k2a],
                        rhs=dgt[:, cl * P:(cl + 1) * P],
                        start=True, stop=True)
                    nc.tensor.matmul(
                        tps[0:k2b, P:2 * P],
                        lhsT=acc[:, cl * FP + k2a:cl * FP + F],
                        rhs=dgt[:, cl * P:(cl + 1) * P],
                        start=True, stop=True)
                    if j % 2 == 0:
                        nc.scalar.activation(fr.pxa[:, jp:jp + P], tps[:, 0:P], AF.Copy)
                        nc.vector.tensor_copy(fr.pxb[:, jp:jp + P], tps[0:k2b, P:2 * P])
                    else:
                        nc.vector.tensor_copy(fr.pxa[:, jp:jp + P], tps[:, 0:P])
                        nc.scalar.activation(fr.pxb[:, jp:jp + P], tps[0:k2b, P:2 * P], AF.Copy)
                return fr

            def stage_back(fr):
                """Matmuls + relu + store for one staged group."""
                g, nw, n0 = fr.g, fr.nw, fr.n0
                sh1g = io.tile([P, G, H], BFD, tag="sh1g")
                for j in range(g):
                    jp = j * P
                    pt = ps.tile([P, H], mybir.dt.float32, tag="pt")
                    nc.tensor.matmul(pt[:], lhsT=fr.ya[:, jp:jp + P], rhs=w10a[:], start=True, stop=False)
                    nc.tensor.matmul(pt[:], lhsT=fr.yb[:, jp:jp + P], rhs=w10b[:], start=False, stop=False)
                    nc.tensor.matmul(pt[:], lhsT=fr.pxa[:, jp:jp + P], rhs=w11a[:], start=False, stop=False)
                    nc.tensor.matmul(pt[:], lhsT=fr.pxb[:, jp:jp + P], rhs=w11b[:], start=False, stop=True)
                    nc.scalar.activation(sh1g[:, j, :], pt[:], AF.Relu)
                sh1_view = sh1_out[n0:n0 + nw, :].rearrange("(g p) h -> p g h", p=P)
                nc.sync.dma_start(out=sh1_view, in_=sh1g[:, :g, :])

            pending = None
            for sb in reversed(range(nsub)):
                c0s = sb * CHS
                c1s = min(CH, c0s + CHS)
                if c0s >= c1s:
                    continue
                acc = accp.tile([P, CHS * FP], BFD, tag="acc")
                _emit_gathers(nc, mybir, acc, table, idx, ks, c0s, c1s, FP)
                dgt = dgp.tile([P, CHS * P], BFD, tag="dg")
                nc.sync.dma_start(out=dgt[:, :(c1s - c0s) * P],
                                  in_=dg_in[:, c0s * P:c1s * P])
                for c0 in range(c0s, c1s, G):
                    g = min(G, c1s - c0)
                    fr = stage_front(sb, c0, g, acc, dgt)
                    if pending is not None:
                        stage_back(pending)
                    pending = fr
            if pending is not None:
                stage_back(pending)
    _Infra.legalize_waits(nc)
    return SpmdKernel(nc, NCORES)


# ----------------------------------------------------------------------------
# launch DE: gather Ph + layer-2 (feature-major) + Wl (node-major)
# ----------------------------------------------------------------------------

def _build_DE(ks, nsub=7, G=4, has_b2=True, has_bl=True):
    """Gather Ph + layer-2 + Wl in the s-scaled domain.

    pm-col(n) = s[n] * (h1@W20 + Ph@W21 + b2)[n]^T built from:
      - rhs sh1T (host-transposed BC output) for the W20 term,
      - transpose-scale of the gathered accumulator by diag(s*nd),
      - outer(b2, s) via a contract-1 matmul (skipped when b2 == 0).
    relu commutes with the positive s scale; the final og stage multiplies
    by rs = 1/s per node partition and adds bl.

    Software-pipelined like BC: transposes+copies for group g are emitted
    before the matmuls of group g-1.
    """
    bass, mybir, tile = _get_mods()
    AL = mybir.AluOpType
    AF = mybir.ActivationFunctionType
    BFD = mybir.dt.bfloat16
    K = sum(ks)
    CHS = (CH + nsub - 1) // nsub
    KT = H // P  # 4
    nc = bass.Bass()
    table = nc.declare_dram_parameter("table", [N + 1, H], BFD, isOutput=False)
    idx_in = nc.declare_dram_parameter("idx", [P, K], mybir.dt.int32, isOutput=False)
    dg_in = nc.declare_dram_parameter("dg", [P, CH * P], BFD, isOutput=False)
    shT = nc.declare_dram_parameter("sh1T", [H, NLOCP], BFD, isOutput=False)
    w20 = nc.declare_dram_parameter("w20", [H, H], BFD, isOutput=False)
    w21 = nc.declare_dram_parameter("w21", [H, H], BFD, isOutput=False)
    wl_in = nc.declare_dram_parameter("wl", [P, KT * C], BFD, isOutput=False)
    rs_in = nc.declare_dram_parameter("rs", [P, CH], mybir.dt.float32, isOutput=False)
    if has_b2:
        b2_in = nc.declare_dram_parameter("b2r", [1, H], BFD, isOutput=False)
        srow_in = nc.declare_dram_parameter("srow", [1, NLOCP], BFD, isOutput=False)
    if has_bl:
        bl_in = nc.declare_dram_parameter("blb", [P, C], mybir.dt.float32, isOutput=False)
    out = nc.declare_dram_parameter("outNM", [NLOCP, C], mybir.dt.float32, isOutput=True)

    class Grp:
        pass

    with tile.TileContext(nc) as tc:
        with tc.tile_pool(name="w", bufs=1) as wp, \
             tc.tile_pool(name="accp", bufs=2) as accp, \
             tc.tile_pool(name="dgp", bufs=2) as dgp, \
             tc.tile_pool(name="io", bufs=3) as io, \
             tc.tile_pool(name="ps", bufs=2, space="PSUM") as ps, \
             tc.tile_pool(name="pst", bufs=3, space="PSUM") as pst, \
             tc.tile_pool(name="pso", bufs=1, space="PSUM") as pso:
            idx = wp.tile([P, K], mybir.dt.int32)
            rs = wp.tile([P, CH], mybir.dt.float32)
            w20t = [[wp.tile([P, P], BFD, name=f"w20_{k}_{i}")
                     for i in range(KT)] for k in range(KT)]
            w21t = [[wp.tile([P, P], BFD, name=f"w21_{k}_{i}")
                     for i in range(KT)] for k in range(KT)]
            wlt = wp.tile([P, KT * C], BFD)
            nc.sync.dma_start(out=idx[:], in_=idx_in[:])
            nc.sync.dma_start(out=rs[:], in_=rs_in[:])
            for k in range(KT):
                for i in range(KT):
                    nc.sync.dma_start(out=w20t[k][i][:], in_=w20[k * P:(k + 1) * P, i * P:(i + 1) * P])
                    nc.sync.dma_start(out=w21t[k][i][:], in_=w21[k * P:(k + 1) * P, i * P:(i + 1) * P])
            nc.sync.dma_start(out=wlt[:], in_=wl_in[:])
            if has_b2:
                b2r = wp.tile([1, H], BFD)
                nc.sync.dma_start(out=b2r[:], in_=b2_in[:])
            if has_bl:
                blb = wp.tile([P, C], mybir.dt.float32)
                nc.sync.dma_start(out=blb[:], in_=bl_in[:])

            def stage_front(sb, c0, g, acc, dgt):
                c0s = sb * CHS
                n0 = c0 * P
                nw = g * P
                NW = G * P
                fr = Grp()
                fr.c0, fr.g, fr.n0, fr.nw = c0, g, n0, nw
                fr.hts = [io.tile([P, NW], BFD, tag=f"ht_{i}", name=f"ht_{i}")
                          for i in range(KT)]
                fr.phts = [io.tile([P, NW], BFD, tag=f"pt_{i}", name=f"pt_{i}")
                           for i in range(KT)]
                for i in range(KT):
                    nc.sync.dma_start(out=fr.hts[i][:, :nw], in_=shT[i * P:(i + 1) * P, n0:n0 + nw])
                if has_b2:
                    fr.srow = io.tile([1, NW], BFD, tag="srow")
                    nc.sync.dma_start(out=fr.srow[:, :nw], in_=srow_in[:, n0:n0 + nw])
                for j in range(g):
                    cl = c0 - c0s + j
                    tps = pst.tile([P, H], mybir.dt.float32, tag="tp")
                    for k in range(KT):
                        nc.tensor.matmul(
                            tps[:, k * P:(k + 1) * P],
                            lhsT=acc[:, (cl * KT + k) * P:(cl * KT + k + 1) * P],
                            rhs=dgt[:, cl * P:(cl + 1) * P],
                            start=True, stop=True)
                    jp = j * P
                    if j % 2 == 0:
                        for k in range(KT):
                            nc.scalar.activation(fr.phts[k][:, jp:jp + P],
                                                 tps[:, k * P:(k + 1) * P], AF.Copy)
                    else:
                        for k in range(KT):
                            nc.vector.tensor_copy(fr.phts[k][:, jp:jp + P],
                                                  tps[:, k * P:(k + 1) * P])
                return fr

            def stage_back(fr):
                g, nw, n0, c0 = fr.g, fr.nw, fr.n0, fr.c0
                NW = G * P
                og = io.tile([P, G, C], mybir.dt.float32, tag="og")
                po = pso.tile([P, G * C], mybir.dt.float32, tag="po")
                h2ts = []
                for i in range(KT):
                    pm = ps.tile([P, NW], mybir.dt.float32, tag="pm")
                    nc.tensor.matmul(pm[:, :nw], lhsT=w20t[0][i][:], rhs=fr.hts[0][:, :nw], start=True, stop=False)
                    for k in range(1, KT):
                        nc.tensor.matmul(pm[:, :nw], lhsT=w20t[k][i][:], rhs=fr.hts[k][:, :nw], start=False, stop=False)
                    for k in range(KT):
                        nc.tensor.matmul(pm[:, :nw], lhsT=w21t[k][i][:], rhs=fr.phts[k][:, :nw],
                                         start=False, stop=(not has_b2 and k == KT - 1))
                    if has_b2:
                        nc.tensor.matmul(pm[:, :nw], lhsT=b2r[:, i * P:(i + 1) * P],
                                         rhs=fr.srow[:, :nw], start=False, stop=True)
                    h2t = io.tile([P, NW], BFD, tag=f"h2t_{i}", name=f"h2t_{i}")
                    h2ts.append(h2t)
                    nc.scalar.activation(h2t[:, :nw], pm[:, :nw], AF.Relu)
                # wl matmuls: each po slice's accumulation group must stay
                # consecutive on PE (interleaving an open start..stop group
                # with other matmuls corrupts PSUM accumulation).
                for j in range(g):
                    jp = j * P
                    for i in range(KT):
                        nc.tensor.matmul(
                            po[:, j * C:(j + 1) * C],
                            lhsT=h2ts[i][:, jp:jp + P],
                            rhs=wlt[:, i * C:(i + 1) * C],
                            start=(i == 0), stop=(i == KT - 1))
                for j in range(g):
                    nc.vector.tensor_scalar(
                        og[:, j, :], po[:, j * C:(j + 1) * C],
                        rs[:, c0 + j:c0 + j + 1], None, AL.mult)
                    if has_bl:
                        nc.vector.tensor_tensor(
                            out=og[:, j, :], in0=og[:, j, :], in1=blb[:], op=AL.add)
                out_view = out[n0:n0 + nw, :].rearrange("(g p) c -> p g c", p=P)
                nc.sync.dma_start(out=out_view, in_=og[:, :g, :])

            pending = None
            for sb in reversed(range(nsub)):
                c0s = sb * CHS
                c1s = min(CH, c0s + CHS)
                if c0s >= c1s:
                    continue
                acc = accp.tile([P, CHS * H], BFD, tag="acc")
                _emit_gathers(nc, mybir, acc, table, idx, ks, c0s, c1s, H)
                dgt = dgp.tile([P, CHS * P], BFD, tag="dg")
                nc.sync.dma_start(out=dgt[:, :(c1s - c0s) * P],
                                  in_=dg_in[:, c0s * P:c1s * P])
                for c0 in range(c0s, c1s, G):
                    g = min(G, c1s - c0)
                    fr = stage_front(sb, c0, g, acc, dgt)
                    if pending is not None:
                        stage_back(pending)
                    pending = fr
            if pending is not None:
                stage_back(pending)
    _Infra.legalize_waits(nc)
    return SpmdKernel(nc, NCORES)


# ----------------------------------------------------------------------------
# main kernel
# ----------------------------------------------------------------------------

TIME_ITERS = 0
LAST_TIMES = {}
LAST_KERNELS = {}


def kernel(x, edge_index, W1_0, W1_1, b1, W2_0, W2_1, b2, Wl, bl):
    x = np.asarray(x, np.float32)
    edge_index = np.asarray(edge_index)
    W1_0 = np.asarray(W1_0, np.float32); W1_1 = np.asarray(W1_1, np.float32)
    b1 = np.asarray(b1, np.float32); W2_0 = np.asarray(W2_0, np.float32)
    W2_1 = np.asarray(W2_1, np.float32); b2 = np.asarray(b2, np.float32)
    Wl = np.asarray(Wl, np.float32); bl = np.asarray(bl, np.float32)
    src = edge_index[0].astype(np.int64)
    dst = edge_index[1].astype(np.int64)

    perms, idx, ks = _host_prep(src, dst)
    deg = np.bincount(src, minlength=N).astype(np.float32)
    has_b2 = bool(np.any(b2))
    has_bl = bool(np.any(bl))

    if "A" not in _CACHE:
        _CACHE["A"] = _build_A()
    kbc_key = ("BC", ks)
    kde_key = ("DE", ks, has_b2, has_bl)
    if kbc_key not in _CACHE:
        _CACHE[kbc_key] = _build_BC(ks)
    if kde_key not in _CACHE:
        _CACHE[kde_key] = _build_DE(ks, has_b2=has_b2, has_bl=has_bl)
    kA, kBC, kDE = _CACHE["A"], _CACHE[kbc_key], _CACHE[kde_key]
    LAST_KERNELS.clear()
    LAST_KERNELS.update({"A": kA, "BC": kBC, "DE": kDE})

    xbf = x.astype(BF)

    # ---- launch A
    in_maps = []
    for p in range(NCORES):
        lo = p * NLOC
        xin = np.zeros((NLOCP, F), BF)
        xin[:NLOC] = xbf[lo:lo + NLOC]
        xcm = np.ascontiguousarray(
            xin.reshape(CH, P, F).transpose(1, 0, 2).reshape(P, CH * F))
        degv = np.zeros(NLOCP, np.float32)
        degv[:NLOC] = deg[lo:lo + NLOC]
        in_maps.append({"xcm": xcm, "deg": _cmajor(degv)})
    pA = kA.place(in_maps)
    resA = kA.run(pA)
    if TIME_ITERS:
        LAST_TIMES["A"] = kA.time_iters(pA, TIME_ITERS)

    # host layout between A and BC (pure permute/transpose/concat/diag)
    table_x = np.zeros((N + 1, FP), BF)
    yranks, s_ranks, rs_rank_cms, dgs = [], [], [], []
    for p in range(NCORES):
        yrows = resA[p]["ycm"].reshape(P, CH, F).transpose(1, 0, 2).reshape(NLOCP, F)
        table_x[p * NLOC:(p + 1) * NLOC, :F] = yrows[:NLOC]
        order_l = perms[p] - p * NLOC
        yrank = np.zeros((NLOCP, F), BF)
        yrank[:NLOC] = yrows[order_l]
        yranks.append(yrank)
        s_flat = resA[p]["s"].T.reshape(NLOCP)
        snd_flat = resA[p]["snd"].T.reshape(NLOCP)
        rs_flat = resA[p]["rs"].T.reshape(NLOCP)
        s_rank = np.zeros(NLOCP, np.float32)
        s_rank[:NLOC] = s_flat[order_l]
        snd_rank = np.zeros(NLOCP, np.float32)
        snd_rank[:NLOC] = snd_flat[order_l]
        rs_rank = np.ones(NLOCP, np.float32)
        rs_rank[:NLOC] = rs_flat[order_l]
        s_ranks.append(s_rank)
        rs_rank_cms.append(_cmajor(rs_rank))
        sndc = _cmajor(snd_rank)              # [P, CH]
        dg = np.zeros((P, CH, P), BF)
        ii = np.arange(P)
        dg[ii, :, ii] = sndc.astype(BF)
        dgs.append(dg.reshape(P, CH * P))

    # ---- launch BC
    w10aug = np.vstack([W1_0, b1[None, :]]).astype(BF)
    in_maps = []
    for p in range(NCORES):
        yaugT = np.empty((FA, NLOCP), BF)
        yaugT[:F] = yranks[p].T
        yaugT[F] = s_ranks[p].astype(BF)
        in_maps.append({
            "table": table_x, "idx": idx[p], "dg": dgs[p],
            "yaugT": yaugT, "w10aug": w10aug, "w11": W1_1.astype(BF),
        })
    pBC = kBC.place(in_maps)
    resBC = kBC.run(pBC)
    if TIME_ITERS:
        LAST_TIMES["BC"] = kBC.time_iters(pBC, TIME_ITERS)

    # host layout between BC and DE
    table_h = np.zeros((N + 1, H), BF)
    for p in range(NCORES):
        table_h[perms[p]] = resBC[p]["sh1"][:NLOC]

    # ---- launch DE
    wlc = np.ascontiguousarray(
        Wl.reshape(H // P, P, C).transpose(1, 0, 2).reshape(P, -1)).astype(BF)
    in_maps = []
    for p in range(NCORES):
        m = {
            "table": table_h, "idx": idx[p], "dg": dgs[p],
            "sh1T": np.ascontiguousarray(resBC[p]["sh1"].T),
            "w20": W2_0.astype(BF), "w21": W2_1.astype(BF),
            "wl": wlc, "rs": rs_rank_cms[p],
        }
        if has_b2:
            m["b2r"] = b2.reshape(1, H).astype(BF)
            m["srow"] = s_ranks[p].reshape(1, NLOCP).astype(BF)
        if has_bl:
            m["blb"] = np.tile(bl[None, :], (P, 1)).astype(np.float32)
        in_maps.append(m)
    pDE = kDE.place(in_maps)
    resDE = kDE.run(pDE)
    if TIME_ITERS:
        LAST_TIMES["DE"] = kDE.time_iters(pDE, TIME_ITERS)

    out = np.empty((N, C), np.float32)
    for p in range(NCORES):
        out[perms[p]] = resDE[p]["outNM"][:NLOC]
    return out
